# revision 8
# baseline (speedup 1.0000x reference)
# Trainium2 Bass kernel for CustomLSTMModel (V=32000, E=256, H=512, O=2, S=512, B=64)
#
# Strategy: data-parallel over batch (8 cores x B_loc=8). Weights replicated and
# SBUF-resident. Embedding rows gathered on-device with a transposed dma_gather
# (feature-major layout). Input projections G_x = W_x @ x_t + b precomputed for
# all timesteps in chunks of 64 steps, interleaved with the recurrence on the PE.
#
# Per-step schedule (v3): PE matmuls are instruction-floor bound (~27ns each
# regardless of dtype), so the burst shrinks only by issuing fewer matmuls:
# prec="fp8dr" uses fp8 DoubleRow (2 k-tiles per instruction, 32 W_h matmuls).
# Group order chat, f, i, o: tanh(chat) runs during f/i groups (also soaking up
# the ACT wake penalty), one sigmoid ACT covers [f|i] right after group 3, the
# c-update is one [128,64] DVE mul + one add, sig(o)/tanh(c_new) overlap the o
# tail. Gate PSUM pools are double-buffered so next-step seeds never stall the
# in-order PE queue on PSUM-reuse waits. G_x copies run on GpSimd to keep DVE
# free for the chain.
#
# fp8 variants store W_h as float8e4 scaled by 64; the 1/64 is folded into the
# gate ACT scale, with G_x and biases pre-scaled by 64 on the host.

import numpy as np
import ml_dtypes

S, B, V, E, H, O = 512, 64, 32000, 256, 512, 2
NCORES = 8
BLOC = B // NCORES          # 8 batch elements per core
CHUNK = 64                  # timesteps per G_x precompute chunk
NCHUNK = S // CHUNK
MT = 16                     # gate-row tiles (4 gates x 4 tiles of 128)
KT = H // 128               # 4 contraction tiles over h
KC = E // 128               # 2 contraction tiles over x
NIDX = S * BLOC             # 4096 tokens gathered per core

FP8_SCALE = 64.0            # W_h pre-scale for fp8 quantization

_CACHE = {}


def _build_bass(prec, n_steps, dummies=0):
    import concourse.bass as bass
    import concourse.bacc as bacc
    import concourse.tile as tile
    import concourse.mybir as mybir
    from contextlib import ExitStack

    AF = mybir.ActivationFunctionType
    fp32 = mybir.dt.float32
    bf16 = mybir.dt.bfloat16
    fp8 = mybir.dt.float8e4
    dt = bf16
    use_fp8 = prec in ("fp8", "fp8dr")
    dr = prec == "fp8dr"
    wh_dt = fp8 if use_fp8 else dt
    h_dt = fp8 if dr else dt
    act_scale = (1.0 / FP8_SCALE) if use_fp8 else 1.0
    DR = mybir.MatmulPerfMode.DoubleRow if dr else None

    nc = bacc.Bacc("TRN2")
    idx_d = nc.declare_dram_parameter("idx", [128, NIDX // 128], mybir.dt.int32, isOutput=False)
    emb_d = nc.declare_dram_parameter("embt", [V, E], fp32, isOutput=False)
    ident_d = nc.declare_dram_parameter("ident", [128, 128], fp32, isOutput=False)
    whT_d = nc.declare_dram_parameter("whT", [128, KT * 2048], wh_dt, isOutput=False)
    wxT_d = nc.declare_dram_parameter("wxT", [128, KC * 2048], dt, isOutput=False)
    bf_d = nc.declare_dram_parameter("bfold", [128, MT], fp32, isOutput=False)
    wy_d = nc.declare_dram_parameter("wyT", [128, KT * O], fp32, isOutput=False)
    by_d = nc.declare_dram_parameter("byT", [1, O], fp32, isOutput=False)
    y_d = nc.declare_dram_parameter("y", [BLOC, O], fp32, isOutput=True)

    with tile.TileContext(nc) as tc, ExitStack() as ctx:
        const = ctx.enter_context(tc.tile_pool(name="const", bufs=1))
        gxp = ctx.enter_context(tc.tile_pool(name="gx", bufs=2))
        hp = ctx.enter_context(tc.tile_pool(name="h", bufs=2))
        stp = ctx.enter_context(tc.tile_pool(name="st", bufs=2))
        wk = ctx.enter_context(tc.tile_pool(name="wk", bufs=2))
        psx = ctx.enter_context(tc.tile_pool(name="psx", bufs=2, space="PSUM"))

        idx_sb = const.tile([128, NIDX // 128], mybir.dt.int32)
        nc.sync.dma_start(idx_sb[:], idx_d[:])
        ident = const.tile([128, 128], fp32)
        nc.sync.dma_start(ident[:], ident_d[:])
        ident_s = const.tile([128, 128], bf16, name="ident_s")
        nc.vector.tensor_copy(ident_s[:], ident[:])
        whT = const.tile([128, KT * 2048], wh_dt)
        nc.sync.dma_start(whT[:], whT_d[:])
        wxT = const.tile([128, KC * 2048], dt)
        nc.sync.dma_start(wxT[:], wxT_d[:])
        bfold = const.tile([128, MT], fp32)
        nc.sync.dma_start(bfold[:], bf_d[:])
        wyT = const.tile([128, KT * O], fp32)
        nc.sync.dma_start(wyT[:], wy_d[:])
        byT = const.tile([1, O], fp32)
        nc.sync.dma_start(byT[:], by_d[:])
        ones = const.tile([1, BLOC], fp32)
        nc.gpsimd.memset(ones[:], 1.0)

        # Gather embedding rows (tokens on partitions), then PE-transpose into the
        # feature-major layout xg[p, c, i] = emb[tok_i, c*128+p], i = t*BLOC + b.
        # The transpose PSUM pool is scoped so its bank frees up before the gate
        # PSUM pools are created (8-bank budget).
        xg = const.tile([128, KC, NIDX], dt)
        with tc.tile_pool(name="gp", bufs=3) as gp, tc.tile_pool(
            name="pst", bufs=1, space="PSUM"
        ) as pst:
            for g in range(NIDX // 128):
                xrows = gp.tile([128, E], fp32, tag="xrows")
                nc.gpsimd.indirect_dma_start(
                    out=xrows[:],
                    out_offset=None,
                    in_=emb_d[:, :],
                    in_offset=bass.IndirectOffsetOnAxis(ap=idx_sb[:, g : g + 1], axis=0),
                )
                for kc in range(KC):
                    pt = pst.tile([128, 128], fp32, tag="pt")
                    nc.tensor.transpose(pt[:], xrows[:, kc * 128 : (kc + 1) * 128], ident[:])
                    eng = nc.vector if (g + kc) % 2 == 0 else nc.scalar
                    if eng is nc.vector:
                        nc.vector.tensor_copy(xg[:, kc, g * 128 : (g + 1) * 128], pt[:])
                    else:
                        nc.scalar.copy(xg[:, kc, g * 128 : (g + 1) * 128], pt[:])

        # Gate PSUM pools. psA/psC double-buffered: their ACT reads (sig_fi,
        # sig_o) land mid/late in the step, so next-step seeds would stall the
        # in-order PE queue on the reuse wait. psB's reader (tanh_chat) runs
        # first thing, so one buffer suffices. 8-bank budget:
        # psx(2) + pst(1, prologue) + psA(2) + psB(1) + psC(2).
        psA = ctx.enter_context(tc.tile_pool(name="psA", bufs=2, space="PSUM"))
        psB = ctx.enter_context(tc.tile_pool(name="psB", bufs=1, space="PSUM"))
        psC = ctx.enter_context(tc.tile_pool(name="psC", bufs=2, space="PSUM"))

        n_chunks = (n_steps + CHUNK - 1) // CHUNK
        gx_tiles = {}

        def emit_gx(c, m):
            # G_x for chunk c, gate-row tile m: one PSUM [128, 64*8] over 2 k-tiles,
            # then bias-add copy (on GpSimd) into the chunk buffer.
            if m == 0:
                gx_tiles[c] = gxp.tile([128, CHUNK * 128], dt, tag="gx", name=f"gx{c}")
            ps = psx.tile([128, CHUNK * BLOC], fp32, tag="psx")
            for kc in range(KC):
                nc.tensor.matmul(
                    ps[:],
                    wxT[:, kc * 2048 + m * 128 : kc * 2048 + (m + 1) * 128],
                    xg[:, kc, c * CHUNK * BLOC : (c + 1) * CHUNK * BLOC],
                    start=(kc == 0),
                    stop=(kc == KC - 1),
                )
            dst = gx_tiles[c][:].rearrange("p (t mm) -> p t mm", t=CHUNK)[:, :, m * BLOC : (m + 1) * BLOC]
            src = ps[:].rearrange("p (t b) -> p t b", t=CHUNK)
            nc.vector.tensor_scalar_add(dst, src, bfold[:, m : m + 1])

        for m in range(MT):
            emit_gx(0, m)

        # Gate order in the folded layout (host side): f=0, i=1, o=2, chat=3.
        # gx columns per step: [f(0:32) | i(32:64) | o(64:96) | chat(96:128)].
        # PE group emission order: chat, f, i, o.
        h_cur = None
        st_cur = None  # [128, 64]: cols 0:32 = c_{t-1}, cols 32:64 = tanh(chat_t)
        NB = KT * BLOC  # 32 columns per gate
        st0 = stp.tile([128, 2 * NB], fp32, tag="st", name="st_init")
        nc.gpsimd.memset(st0[:, 0:NB], 0.0)
        st_cur = st0

        whT_k = whT[:].rearrange("p (k x) -> p k x", k=KT)

        def wh_group(g, dst, off):
            for j in range(KT):
                m = g * 4 + j
                if dr:
                    for P in range(KT // 2):
                        nc.tensor.matmul(
                            dst[:, off + j * BLOC : off + (j + 1) * BLOC],
                            whT_k[:, 2 * P : 2 * P + 2, m * 128 : (m + 1) * 128],
                            h_cur[:, 2 * P * BLOC : (2 * P + 2) * BLOC].rearrange(
                                "p (k b) -> p k b", k=2
                            ),
                            start=False,
                            stop=(j == KT - 1 and P == KT // 2 - 1),
                            perf_mode=DR,
                        )
                else:
                    for k in range(KT):
                        nc.tensor.matmul(
                            dst[:, off + j * BLOC : off + (j + 1) * BLOC],
                            whT[:, k * 2048 + m * 128 : (k * 2048 + (m + 1) * 128)],
                            h_cur[:, k * BLOC : (k + 1) * BLOC],
                            start=False,
                            stop=(j == KT - 1 and k == KT - 1),
                        )

        for t in range(n_steps):
            c = t // CHUNK
            tl = t % CHUNK

            pA = psA.tile([128, 2 * NB], fp32, tag="psA", name=f"pA_{t}")
            pB = psB.tile([128, NB], fp32, tag="psB", name=f"pB_{t}")
            pC = psC.tile([128, NB], fp32, tag="psC", name=f"pC_{t}")
            # Seeds: G_x[t] into the gate PSUM banks. gx col layout f|i|o|chat.
            gxt = gx_tiles[c][:, tl * 128 : (tl + 1) * 128]
            nc.tensor.matmul(pB[:], ident_s[:], gxt[:, 96:128], start=True, stop=(t == 0))
            nc.tensor.matmul(pA[:], ident_s[:], gxt[:, 0:64], start=True, stop=(t == 0))
            nc.tensor.matmul(pC[:], ident_s[:], gxt[:, 64:96], start=True, stop=(t == 0))
            # interleave next chunk's input-projection matmuls into the tail window
            if c + 1 < n_chunks and tl < 2 * MT and tl % 2 == 0:
                emit_gx(c + 1, tl // 2)
            if dummies:
                dmy_ps = psx.tile([128, CHUNK * BLOC], fp32, tag="psx", name=f"dmy_{t}")
                for dmy in range(dummies):
                    nc.tensor.matmul(
                        dmy_ps[0:1, 0:1],
                        ident_s[:, 0:1],
                        ident_s[:, 1:2],
                        start=True,
                        stop=True,
                        skip_group_check=True,
                    )

            # W_h @ h: group order chat, f, i, o. PSUM dst per gate:
            # chat -> pB, f -> pA[:, 0:], i -> pA[:, 32:], o -> pC.
            if t > 0:
                for g, dst, off in ((3, pB, 0), (0, pA, 0), (1, pA, NB), (2, pC, 0)):
                    wh_group(g, dst, off)

            gact = wk.tile([128, 3 * NB], fp32, tag="gact")
            st_new = stp.tile([128, 2 * NB], fp32, tag="st", name=f"st_{t}")
            tt = wk.tile([128, 2 * NB], fp32, tag="tt")
            tcv = wk.tile([128, NB], fp32, tag="tc")
            h_new = hp.tile([128, NB], h_dt, tag="h")

            # tanh(chat) first (group 1), into the state tile next to c.
            nc.scalar.activation(st_cur[:, NB : 2 * NB], pB[:], AF.Tanh, scale=act_scale)
            # sig over [f|i] in one ACT as soon as group 3 is done.
            nc.scalar.activation(gact[:, 0 : 2 * NB], pA[:], AF.Sigmoid, scale=act_scale)
            # c update: one [128,64] mul + one add.
            nc.vector.tensor_mul(tt[:], gact[:, 0 : 2 * NB], st_cur[:])
            nc.vector.tensor_add(st_new[:, 0:NB], tt[:, 0:NB], tt[:, NB : 2 * NB])
            nc.scalar.activation(gact[:, 2 * NB : 3 * NB], pC[:], AF.Sigmoid, scale=act_scale)
            nc.scalar.activation(tcv[:], st_new[:, 0:NB], AF.Tanh)
            nc.vector.tensor_mul(h_new[:], gact[:, 2 * NB : 3 * NB], tcv[:])
            if t == n_steps - 1:
                h_fin = wk.tile([128, NB], fp32, tag="hfin")
                nc.vector.tensor_mul(h_fin[:], gact[:, 2 * NB : 3 * NB], tcv[:])
            h_cur, st_cur = h_new, st_new

        # y = h^T @ Wy^T + by  (fp32), reusing a psx bank region
        psy_t = psx.tile([128, CHUNK * BLOC], fp32, tag="psx", name="psy")[0:BLOC, 0:O]
        for j in range(KT):
            nc.tensor.matmul(
                psy_t[:],
                h_fin[:, j * BLOC : (j + 1) * BLOC],
                wyT[:, j * O : (j + 1) * O],
                start=(j == 0),
                stop=False,
            )
        nc.tensor.matmul(psy_t[:], ones[:], byT[:], start=False, stop=True)
        y_sb = wk.tile([BLOC, O], fp32, tag="ysb")
        nc.vector.tensor_copy(y_sb[:], psy_t[:])
        nc.sync.dma_start(y_d[:], y_sb[:])

    nc.compile()
    return nc


def _prep_inputs(texts, emb, Wf, bf, Wi, bi, Wo, bo, Wc, bc, Wy, by, prec):
    bf16 = ml_dtypes.bfloat16
    use_fp8 = prec.startswith("fp8")

    texts = np.asarray(texts)
    emb = np.asarray(emb, dtype=np.float32)

    # Host gate order: f, i, o, chat.
    Wall = np.concatenate(
        [np.asarray(Wf), np.asarray(Wi), np.asarray(Wo), np.asarray(Wc)], axis=0
    ).astype(np.float32)  # [2048, 768]
    Wh = Wall[:, :H]      # [2048, 512] multiplies h
    Wx = Wall[:, H:]      # [2048, 256] multiplies x
    ball = np.concatenate(
        [np.asarray(bf), np.asarray(bi), np.asarray(bo), np.asarray(bc)], axis=0
    ).astype(np.float32).reshape(-1)  # [2048]

    if use_fp8:
        fp8 = ml_dtypes.float8_e4m3fn
        whT = np.ascontiguousarray(
            (Wh * FP8_SCALE).reshape(MT, 128, KT, 128).transpose(3, 2, 0, 1).reshape(128, KT * 2048)
        ).astype(fp8)
        gx_scale = FP8_SCALE
    else:
        whT = np.ascontiguousarray(
            Wh.reshape(MT, 128, KT, 128).transpose(3, 2, 0, 1).reshape(128, KT * 2048)
        ).astype(bf16)
        gx_scale = 1.0
    wxT = np.ascontiguousarray(
        (Wx * gx_scale).reshape(MT, 128, KC, 128).transpose(3, 2, 0, 1).reshape(128, KC * 2048)
    ).astype(bf16)
    bfold = np.ascontiguousarray((ball * gx_scale).reshape(MT, 128).T).astype(np.float32)  # [128, 16]
    Wy = np.asarray(Wy, dtype=np.float32)  # [2, 512]
    wyT = np.ascontiguousarray(Wy.reshape(O, KT, 128).transpose(2, 1, 0).reshape(128, KT * O)).astype(np.float32)
    byT = np.asarray(by, dtype=np.float32).reshape(1, O)

    ident = np.eye(128, dtype=np.float32)

    per_core = []
    for ci in range(NCORES):
        sl = texts[:, ci * BLOC : (ci + 1) * BLOC]  # [S, BLOC]
        flat = np.ascontiguousarray(sl).reshape(-1).astype(np.int64)  # i = t*BLOC + b
        idx = np.ascontiguousarray(flat.astype(np.int32).reshape(NIDX // 128, 128).T)
        per_core.append(
            {
                "idx": idx,
                "embt": emb,
                "ident": ident,
                "whT": whT,
                "wxT": wxT,
                "bfold": bfold,
                "wyT": wyT,
                "byT": byT,
            }
        )
    return per_core


def _parse_prec(prec):
    # "bf16", "fp8", "fp8dr", optionally with "+dN" dummy-fill suffix
    dummies = 0
    if "+d" in prec:
        prec, dd = prec.split("+d")
        dummies = int(dd)
    return prec, dummies


def _get_nc(prec="fp8dr", n_steps=S):
    key = (prec, n_steps)
    if key not in _CACHE:
        base, dummies = _parse_prec(prec)
        _CACHE[key] = _build_bass(base, n_steps, dummies=dummies)
    return _CACHE[key]


def kernel(**inputs):
    prec = inputs.pop("_prec", "fp8dr")
    from concourse.bass_utils import run_bass_kernel_spmd

    nc = _get_nc(prec)
    base, _ = _parse_prec(prec)
    in_maps = _prep_inputs(
        inputs["texts"], inputs["emb"],
        inputs["Wf"], inputs["bf"], inputs["Wi"], inputs["bi"],
        inputs["Wo"], inputs["bo"], inputs["Wc"], inputs["bc"],
        inputs["Wy"], inputs["by"], base,
    )
    res = run_bass_kernel_spmd(nc, in_maps, list(range(NCORES)))
    y = np.concatenate([np.asarray(res.results[i]["y"]) for i in range(NCORES)], axis=0)
    return y.astype(np.float32)
